# revision 1
# baseline (speedup 1.0000x reference)
"""ComplexAttentionV3 Trainium2 kernel (v2).

Sharding: 8 cores = data-parallel over batch (2) x tensor-parallel over
heads (16 -> 4 per core). Each core computes q/k/v for its 4 heads
(column-sharded projections), local attention, and a row-sharded
o-projection producing a partial [T, D] output; the host sums the 4
partials per batch.

v2 notes vs v1: input DMAs split across both HWDGE queues (SP + ACT) and
ordered so the PE can start within ~1 MB of traffic; q/k projections run
before v (they consume x incrementally); attention uses 2-bank-wide PSUM
tiles so exp/reciprocal/normalize run half as many, twice as large ops;
v-projection streams a packed [wvr | wvi] rhs (halves its matmul count);
softmax reciprocal uses the fast custom-DVE approximation straight out
of PSUM; PSUM pools are per-phase and released between phases.
"""

import numpy as np
import ml_dtypes

import concourse.bacc as bacc
import concourse.tile as tile
from concourse import mybir
from concourse.bass import ts
from concourse.bass_utils import run_bass_kernel_spmd

B, T, D, H = 2, 2048, 1024, 16
HD = 64
NCORE = 8
TP = 4               # head-parallel degree (per batch)
HC = H // TP         # heads per core = 4
C = HC * HD          # local channels = 256
DC = D // 128        # contraction chunks = 8
TQ = T // 128        # 128-row t-chunks = 16
T5 = T // 512        # 512-col t-chunks = 4
TW = T // 1024       # 1024-col t-chunks = 2

F32 = mybir.dt.float32
BF16 = mybir.dt.bfloat16
EXP = mybir.ActivationFunctionType.Exp

LAST_RESULTS = None
_COMPILED = None


def _build():
    nc = bacc.Bacc("TRN2", target_bir_lowering=False, debug=False,
                   num_devices=NCORE)

    def din(name, shape, dt=BF16):
        return nc.dram_tensor(name, shape, dt, kind="ExternalInput").ap()

    xr_d = din("xrT", [128, DC, T])
    xi_d = din("xiT", [128, DC, T])
    wq = {k: din(f"wq_{k}", [128, DC, C]) for k in ("r", "i", "n")}
    wk = {k: din(f"wk_{k}", [128, DC, C]) for k in ("r", "i", "n")}
    wv = {k: din(f"wv_{k}", [128, DC, 2 * C]) for k in ("a", "b")}
    ow = {k: din(f"ow_{k}", [128, 2, D]) for k in ("r", "i", "n")}
    cos_d = din("cos2", [128, T], F32)
    sin_d = din("sin2", [128, T], F32)
    outr_d = nc.dram_tensor("out_r", [T, D], F32, kind="ExternalOutput").ap()
    outi_d = nc.dram_tensor("out_i", [T, D], F32, kind="ExternalOutput").ap()

    with tile.TileContext(nc) as tc:
        with tc.tile_pool(name="persist", bufs=1) as persist:
            qkcat = persist.tile([128, 2 * HC, T], BF16, name="qkcat")
            vcat = persist.tile([128, TQ, HC, 128], BF16, name="vcat")
            urt = persist.tile([128, 2, T], BF16, name="urt")
            uit = persist.tile([128, 2, T], BF16, name="uit")
            ones = persist.tile([128, 1], BF16, name="ones")
            nc.vector.memset(ones[:], 1.0)

            # ---------------- projection phase ----------------
            with tc.tile_pool(name="xw", bufs=1) as xw, \
                 tc.tile_pool(name="rt", bufs=1) as rt, \
                 tc.tile_pool(name="pp", bufs=2, space="PSUM") as pp:
                # weights + rope tables ride the ACT HWDGE queue, x rides
                # SP: the first q matmul needs only wq + xr[dc0].
                wqs = {k: xw.tile([128, DC, C], BF16, name=f"wq{k}")
                       for k in ("r", "i", "n")}
                wks = {k: xw.tile([128, DC, C], BF16, name=f"wk{k}")
                       for k in ("r", "i", "n")}
                wvs = {k: xw.tile([128, DC, 2 * C], BF16, name=f"wv{k}")
                       for k in ("a", "b")}
                cos = xw.tile([128, T], F32, name="cos")
                sin = xw.tile([128, T], F32, name="sin")
                for k in ("r", "i", "n"):
                    nc.scalar.dma_start(wqs[k][:], wq[k][:])
                    nc.scalar.dma_start(wks[k][:], wk[k][:])
                for k in ("a", "b"):
                    nc.scalar.dma_start(wvs[k][:], wv[k][:])
                nc.scalar.dma_start(cos[:], cos_d[:])
                nc.scalar.dma_start(sin[:], sin_d[:])
                xr = xw.tile([128, DC, T], BF16, name="xr")
                xi = xw.tile([128, DC, T], BF16, name="xi")
                for dc in range(DC):
                    nc.sync.dma_start(xr[:, dc, :], xr_d[:, dc, :])
                    nc.sync.dma_start(xi[:, dc, :], xi_d[:, dc, :])

                # q/k projections (transposed [c, t]) + RoPE into qkcat
                for wsrc, hbase in ((wqs, 0), (wks, HC)):
                    for cc in range(2):
                        h0, h1 = hbase + 2 * cc, hbase + 2 * cc + 1
                        for tw in range(TW):
                            pqr = pp.tile([128, 1024], F32, name="ppa")
                            pqi = pp.tile([128, 1024], F32, name="ppb")
                            for half in range(2):
                                hsl = ts(2 * tw + half, 512)
                                psl = ts(half, 512)
                                for dc in range(DC):
                                    nc.tensor.matmul(
                                        pqr[:, psl],
                                        lhsT=wsrc["r"][:, dc, ts(cc, 128)],
                                        rhs=xr[:, dc, hsl],
                                        start=(dc == 0), stop=False)
                                for dc in range(DC):
                                    nc.tensor.matmul(
                                        pqr[:, psl],
                                        lhsT=wsrc["n"][:, dc, ts(cc, 128)],
                                        rhs=xi[:, dc, hsl],
                                        start=False, stop=(dc == DC - 1))
                                for dc in range(DC):
                                    nc.tensor.matmul(
                                        pqi[:, psl],
                                        lhsT=wsrc["i"][:, dc, ts(cc, 128)],
                                        rhs=xr[:, dc, hsl],
                                        start=(dc == 0), stop=False)
                                for dc in range(DC):
                                    nc.tensor.matmul(
                                        pqi[:, psl],
                                        lhsT=wsrc["r"][:, dc, ts(cc, 128)],
                                        rhs=xi[:, dc, hsl],
                                        start=False, stop=(dc == DC - 1))
                            tsl = ts(tw, 1024)
                            t1 = rt.tile([128, 1024], F32, name="t1")
                            t2 = rt.tile([128, 1024], F32, name="t2")
                            t3 = rt.tile([128, 1024], F32, name="t3")
                            t4 = rt.tile([128, 1024], F32, name="t4")
                            nc.vector.tensor_mul(t1[:], pqr[:], cos[:, tsl])
                            nc.vector.tensor_mul(t2[:], pqi[:], sin[:, tsl])
                            nc.vector.tensor_mul(t3[:], pqr[:], sin[:, tsl])
                            nc.vector.tensor_mul(t4[:], pqi[:], cos[:, tsl])
                            nc.vector.tensor_sub(qkcat[0:64, h0, tsl],
                                                 t1[0:64, :], t2[0:64, :])
                            nc.vector.tensor_sub(qkcat[0:64, h1, tsl],
                                                 t1[64:128, :], t2[64:128, :])
                            nc.vector.tensor_add(qkcat[64:128, h0, tsl],
                                                 t3[0:64, :], t4[0:64, :])
                            nc.vector.tensor_add(qkcat[64:128, h1, tsl],
                                                 t3[64:128, :], t4[64:128, :])

                # v projection: natural [t, c], rhs packed [wvr | wvi]
                for tq in range(TQ):
                    pv = pp.tile([128, 1024], F32, name="ppa")
                    pvs = pv[:, 0:512]
                    for dc in range(DC):
                        nc.tensor.matmul(pvs, lhsT=xr[:, dc, ts(tq, 128)],
                                         rhs=wvs["a"][:, dc, :],
                                         start=(dc == 0), stop=False)
                    for dc in range(DC):
                        nc.tensor.matmul(pvs, lhsT=xi[:, dc, ts(tq, 128)],
                                         rhs=wvs["b"][:, dc, :],
                                         start=False, stop=(dc == DC - 1))
                    nc.scalar.copy(
                        vcat[:, tq, :, 0:64],
                        pv[:, 0:C].rearrange("p (h d) -> p h d", h=HC))
                    nc.scalar.copy(
                        vcat[:, tq, :, 64:128],
                        pv[:, C:2 * C].rearrange("p (h d) -> p h d", h=HC))

            # ---------------- attention phase ----------------
            with tc.tile_pool(name="att", bufs=4) as att, \
                 tc.tile_pool(name="attsm", bufs=2) as attsm, \
                 tc.tile_pool(name="sp", bufs=2, space="PSUM") as sp, \
                 tc.tile_pool(name="avp", bufs=1, space="PSUM") as avp, \
                 tc.tile_pool(name="dp", bufs=1, space="PSUM") as dp:
                for h in range(HC):
                    ucc, up0 = h // 2, (h % 2) * 64
                    for iw in range(TW):
                        isl = ts(iw, 1024)
                        av = avp.tile([128, 1024], F32, name="av")
                        dn = dp.tile([1, 1024], F32, name="dn")
                        for jc in range(TQ):
                            s = sp.tile([128, 1024], F32, name="s")
                            for half in range(2):
                                psl = ts(half, 512)
                                nc.tensor.matmul(
                                    s[:, psl],
                                    lhsT=qkcat[:, HC + h, ts(jc, 128)],
                                    rhs=qkcat[:, h, ts(2 * iw + half, 512)],
                                    start=True, stop=True)
                            es = att.tile([128, 1024], BF16, name="es")
                            nc.scalar.activation(es[:], s[:], EXP, scale=0.125)
                            for half in range(2):
                                psl = ts(half, 512)
                                nc.tensor.matmul(av[:, psl],
                                                 lhsT=vcat[:, jc, h, :],
                                                 rhs=es[:, psl],
                                                 start=(jc == 0),
                                                 stop=(jc == TQ - 1))
                                nc.tensor.matmul(dn[:, psl], lhsT=ones[:],
                                                 rhs=es[:, psl],
                                                 start=(jc == 0),
                                                 stop=(jc == TQ - 1))
                        rec = attsm.tile([1, 1024], F32, name="rec")
                        nc.vector.reciprocal_approx_fast(rec[:], dn[:])
                        bc = attsm.tile([128, 1024], F32, name="bc")
                        nc.gpsimd.partition_broadcast(bc[:], rec[:])
                        nc.vector.tensor_mul(urt[up0:up0 + 64, ucc, isl],
                                             av[0:64, :], bc[0:64, :])
                        nc.vector.tensor_mul(uit[up0:up0 + 64, ucc, isl],
                                             av[64:128, :], bc[64:128, :])

            # ---------------- output projection ----------------
            with tc.tile_pool(name="ox", bufs=1) as ox, \
                 tc.tile_pool(name="ost", bufs=3) as ost, \
                 tc.tile_pool(name="op", bufs=2, space="PSUM") as op:
                ows = {k: ox.tile([128, 2, D], BF16, name=f"ow{k}")
                       for k in ("r", "i", "n")}
                for k in ("r", "i", "n"):
                    nc.scalar.dma_start(ows[k][:], ow[k][:])
                for tq in range(TQ):
                    tslq = ts(tq, 128)
                    por = op.tile([128, 1024], F32, name="opa")
                    poi = op.tile([128, 1024], F32, name="opb")
                    for oc in range(2):
                        osl = ts(oc, 512)
                        nc.tensor.matmul(por[:, osl], lhsT=urt[:, 0, tslq],
                                         rhs=ows["r"][:, 0, osl],
                                         start=True, stop=False)
                        nc.tensor.matmul(por[:, osl], lhsT=urt[:, 1, tslq],
                                         rhs=ows["r"][:, 1, osl],
                                         start=False, stop=False)
                        nc.tensor.matmul(por[:, osl], lhsT=uit[:, 0, tslq],
                                         rhs=ows["n"][:, 0, osl],
                                         start=False, stop=False)
                        nc.tensor.matmul(por[:, osl], lhsT=uit[:, 1, tslq],
                                         rhs=ows["n"][:, 1, osl],
                                         start=False, stop=True)
                        nc.tensor.matmul(poi[:, osl], lhsT=urt[:, 0, tslq],
                                         rhs=ows["i"][:, 0, osl],
                                         start=True, stop=False)
                        nc.tensor.matmul(poi[:, osl], lhsT=urt[:, 1, tslq],
                                         rhs=ows["i"][:, 1, osl],
                                         start=False, stop=False)
                        nc.tensor.matmul(poi[:, osl], lhsT=uit[:, 0, tslq],
                                         rhs=ows["r"][:, 0, osl],
                                         start=False, stop=False)
                        nc.tensor.matmul(poi[:, osl], lhsT=uit[:, 1, tslq],
                                         rhs=ows["r"][:, 1, osl],
                                         start=False, stop=True)
                    st = ost.tile([128, 1024], F32, name="st")
                    nc.scalar.copy(st[:], por[:])
                    nc.sync.dma_start(outr_d[tslq, :], st[:])
                    sti = ost.tile([128, 1024], F32, name="sti")
                    nc.scalar.copy(sti[:], poi[:])
                    nc.sync.dma_start(outi_d[tslq, :], sti[:])

    nc.compile()
    return nc


def _to_bf16_kxm(arr, parts=128):
    """[K, M] fp32 -> [128, K//128, M] bf16 with K split as (chunk, part)."""
    k, m = arr.shape
    out = arr.reshape(k // parts, parts, m).transpose(1, 0, 2)
    return np.ascontiguousarray(out.astype(ml_dtypes.bfloat16))


def _rope_tables():
    inv_freq = 1.0 / (10000.0 ** (np.arange(0, HD, 2, dtype=np.float64) / HD))
    invf64 = np.concatenate([inv_freq, inv_freq])          # [64]
    ang = invf64[:, None] * np.arange(T, dtype=np.float64)[None, :]  # [64, T]
    cos2 = np.tile(np.cos(ang), (2, 1)).astype(np.float32)
    sin2 = np.tile(np.sin(ang), (2, 1)).astype(np.float32)
    return np.ascontiguousarray(cos2), np.ascontiguousarray(sin2)


def kernel(x_real, x_imag, q_wr, q_wi, k_wr, k_wi, v_wr, v_wi, o_wr, o_wi):
    global _COMPILED, LAST_RESULTS
    if _COMPILED is None:
        _COMPILED = _build()
    nc = _COMPILED

    cos2, sin2 = _rope_tables()
    xt = {}
    for b in range(B):
        xt[("r", b)] = _to_bf16_kxm(np.asarray(x_real[b]).T.astype(np.float32))
        xt[("i", b)] = _to_bf16_kxm(np.asarray(x_imag[b]).T.astype(np.float32))

    in_maps = []
    for core in range(NCORE):
        b, g = core // TP, core % TP
        cols = slice(g * C, (g + 1) * C)
        m = {"xrT": xt[("r", b)], "xiT": xt[("i", b)],
             "cos2": cos2, "sin2": sin2}
        for nm, wr_, wi_ in (("wq", q_wr, q_wi), ("wk", k_wr, k_wi)):
            m[f"{nm}_r"] = _to_bf16_kxm(np.asarray(wr_[:, cols]))
            m[f"{nm}_i"] = _to_bf16_kxm(np.asarray(wi_[:, cols]))
            m[f"{nm}_n"] = _to_bf16_kxm(-np.asarray(wi_[:, cols]))
        vr_, vi_ = np.asarray(v_wr[:, cols]), np.asarray(v_wi[:, cols])
        m["wv_a"] = _to_bf16_kxm(np.concatenate([vr_, vi_], axis=1))
        m["wv_b"] = _to_bf16_kxm(np.concatenate([-vi_, vr_], axis=1))
        m["ow_r"] = _to_bf16_kxm(np.asarray(o_wr[cols, :]))
        m["ow_i"] = _to_bf16_kxm(np.asarray(o_wi[cols, :]))
        m["ow_n"] = _to_bf16_kxm(-np.asarray(o_wi[cols, :]))
        in_maps.append(m)

    res = run_bass_kernel_spmd(nc, in_maps, core_ids=list(range(NCORE)))
    LAST_RESULTS = res

    final_r = np.zeros((B, T, D), np.float32)
    final_i = np.zeros((B, T, D), np.float32)
    for core in range(NCORE):
        b = core // TP
        final_r[b] += res.results[core]["out_r"]
        final_i[b] += res.results[core]["out_i"]
    return final_r, final_i



# revision 8
# speedup vs baseline: 1.0570x; 1.0570x over previous
"""ComplexAttentionV3 Trainium2 kernel (v3).

Sharding: 8 cores = data-parallel over batch (2) x tensor-parallel over
heads (16 -> 4 per core). Each core computes q/k/v for its 4 heads
(column-sharded projections), local attention, and a row-sharded
o-projection producing a partial [T, D] output; the host sums the 4
partials per batch.

v3 notes vs v2:
- All four complex projections use the 3-multiplication (Karatsuba)
  form: M1 = (xr+xi)@wr, M2 = xi@(wr+wi), M3 = xr@(wi-wr);
  real = M1-M2, imag = M1+M3. Weight combos precomputed on host;
  xr+xi computed once on gpsimd/vector and shared by q/k/v.
- x arrives in 4 column-blocks of 512 (host layout [128, T5, DC, 512])
  with 2-deep buffer rotation, so the PE starts ~3us in and x DMA
  hides under projection compute.
- Softmax denominators no longer run on the PE: es chunks are
  accumulated on vector (even jc) and gpsimd (odd jc) in bf16, then
  reduced over partitions with 4 small ones-matmuls per (h, iw).
- Attention is software-pipelined (s for jc+1 issued before av for jc,
  av pool double-buffered) so the PE rides through the exp latency.
- o-projection weights prefetch on the idle sync DMA ring at attention
  start; output DMAs alternate between both HWDGE rings.
"""

import numpy as np
import ml_dtypes

import concourse.bacc as bacc
import concourse.tile as tile
from concourse import mybir
from concourse.bass import ts
from concourse.bass_utils import run_bass_kernel_spmd

B, T, D, H = 2, 2048, 1024, 16
HD = 64
NCORE = 8
TP = 4               # head-parallel degree (per batch)
HC = H // TP         # heads per core = 4
C = HC * HD          # local channels = 256
DC = D // 128        # contraction chunks = 8
TQ = T // 128        # 128-row t-chunks = 16
T5 = T // 512        # 512-col t-chunks = 4
TW = T // 1024       # 1024-col t-chunks = 2

F32 = mybir.dt.float32
BF16 = mybir.dt.bfloat16
EXP = mybir.ActivationFunctionType.Exp

LAST_RESULTS = None
_COMPILED = None


def _build():
    nc = bacc.Bacc("TRN2", target_bir_lowering=False, debug=False,
                   num_devices=NCORE)

    def din(name, shape, dt=BF16):
        return nc.dram_tensor(name, shape, dt, kind="ExternalInput").ap()

    xr_d = din("xrT", [128, T5, DC, 512])
    xi_d = din("xiT", [128, T5, DC, 512])
    # Karatsuba combos: r = w_r, a = w_r + w_i, b = w_i - w_r
    wq = {k: din(f"wq_{k}", [128, DC, C]) for k in ("r", "a", "b")}
    wk = {k: din(f"wk_{k}", [128, DC, C]) for k in ("r", "a", "b")}
    wv = {k: din(f"wv_{k}", [128, DC, C]) for k in ("r", "a", "b")}
    ow = {k: din(f"ow_{k}", [128, 2, D]) for k in ("r", "a", "b")}
    cos_d = din("cos2", [128, T])
    sin_d = din("sin2", [128, T])
    outr_d = nc.dram_tensor("out_r", [T, D], F32, kind="ExternalOutput").ap()
    outi_d = nc.dram_tensor("out_i", [T, D], F32, kind="ExternalOutput").ap()

    with tile.TileContext(nc) as tc:
        with tc.tile_pool(name="persist", bufs=1) as persist:
            qkcat = persist.tile([128, 2 * HC, T], BF16, name="qkcat")
            vcat = persist.tile([128, TQ, HC, 128], BF16, name="vcat")
            urt = persist.tile([128, 2, T], BF16, name="urt")
            uit = persist.tile([128, 2, T], BF16, name="uit")
            ones = persist.tile([128, 1], BF16, name="ones")
            nc.vector.memset(ones[:], 1.0)

            # ---------------- projection phase ----------------
            with tc.tile_pool(name="wts", bufs=1) as wts, \
                 tc.tile_pool(name="xblk", bufs=2) as xblk, \
                 tc.tile_pool(name="rt", bufs=2) as rt, \
                 tc.tile_pool(name="pp", bufs=2, space="PSUM") as pp:
                wqs = {k: wts.tile([128, DC, C], BF16, name=f"wq{k}")
                       for k in ("r", "a", "b")}
                wks = {k: wts.tile([128, DC, C], BF16, name=f"wk{k}")
                       for k in ("r", "a", "b")}
                wvs = {k: wts.tile([128, DC, C], BF16, name=f"wv{k}")
                       for k in ("r", "a", "b")}
                cos = wts.tile([128, T], BF16, name="cos")
                sin = wts.tile([128, T], BF16, name="sin")
                # weights + tables on the ACT ring, ordered by first use
                for k in ("a", "b", "r"):
                    nc.scalar.dma_start(wqs[k][:], wq[k][:])
                nc.scalar.dma_start(cos[:], cos_d[:])
                nc.scalar.dma_start(sin[:], sin_d[:])
                for k in ("a", "b", "r"):
                    nc.scalar.dma_start(wks[k][:], wk[k][:])
                for k in ("a", "b", "r"):
                    nc.scalar.dma_start(wvs[k][:], wv[k][:])

                for t5 in range(T5):
                    xi_t = xblk.tile([128, DC, 512], BF16, name="xi")
                    xr_t = xblk.tile([128, DC, 512], BF16, name="xr")
                    sx_t = xblk.tile([128, DC, 512], BF16, name="sx")
                    nc.sync.dma_start(xi_t[:], xi_d[:, t5])
                    nc.sync.dma_start(xr_t[:], xr_d[:, t5])
                    if t5 == 0:
                        nc.vector.tensor_add(sx_t[:], xr_t[:], xi_t[:])
                    else:
                        nc.gpsimd.tensor_add(sx_t[:], xr_t[:], xi_t[:])
                    tsl = ts(t5, 512)

                    # q/k transposed [c, t] + RoPE into qkcat
                    for wsrc, hbase in ((wqs, 0), (wks, HC)):
                        for cc in range(2):
                            h0, h1 = hbase + 2 * cc, hbase + 2 * cc + 1
                            csl = ts(cc, 128)
                            trio = pp.tile([128, 1536], F32, name="trio")
                            m2, m3, m1 = (trio[:, 0:512], trio[:, 512:1024],
                                          trio[:, 1024:1536])
                            for dc in range(DC):
                                nc.tensor.matmul(
                                    m2, lhsT=wsrc["a"][:, dc, csl],
                                    rhs=xi_t[:, dc, :],
                                    start=(dc == 0), stop=(dc == DC - 1))
                            for dc in range(DC):
                                nc.tensor.matmul(
                                    m3, lhsT=wsrc["b"][:, dc, csl],
                                    rhs=xr_t[:, dc, :],
                                    start=(dc == 0), stop=(dc == DC - 1))
                            for dc in range(DC):
                                nc.tensor.matmul(
                                    m1, lhsT=wsrc["r"][:, dc, csl],
                                    rhs=sx_t[:, dc, :],
                                    start=(dc == 0), stop=(dc == DC - 1))
                            m1s = rt.tile([128, 512], BF16, name="m1s")
                            nc.scalar.copy(m1s[:], m1)
                            tr = rt.tile([128, 512], BF16, name="tr")
                            ti = rt.tile([128, 512], BF16, name="ti")
                            nc.vector.tensor_sub(tr[:], m1s[:], m2)
                            nc.vector.tensor_add(ti[:], m1s[:], m3)
                            t1 = rt.tile([128, 512], BF16, name="t1")
                            t2 = rt.tile([128, 512], BF16, name="t2")
                            t3 = rt.tile([128, 512], BF16, name="t3")
                            t4 = rt.tile([128, 512], BF16, name="t4")
                            nc.vector.tensor_mul(t1[:], tr[:], cos[:, tsl])
                            nc.vector.tensor_mul(t2[:], ti[:], sin[:, tsl])
                            nc.vector.tensor_mul(t3[:], tr[:], sin[:, tsl])
                            nc.vector.tensor_mul(t4[:], ti[:], cos[:, tsl])
                            nc.vector.tensor_sub(qkcat[0:64, h0, tsl],
                                                 t1[0:64, :], t2[0:64, :])
                            nc.vector.tensor_sub(qkcat[0:64, h1, tsl],
                                                 t1[64:128, :], t2[64:128, :])
                            nc.vector.tensor_add(qkcat[64:128, h0, tsl],
                                                 t3[0:64, :], t4[0:64, :])
                            nc.vector.tensor_add(qkcat[64:128, h1, tsl],
                                                 t3[64:128, :], t4[64:128, :])

                    # v natural [t, c] for the 4 tq chunks of this t5
                    for tql in range(4):
                        tq = 4 * t5 + tql
                        qsl = ts(tql, 128)
                        vt = pp.tile([128, 1536], F32, name="trio")
                        v2, v3, v1 = (vt[:, 0:256], vt[:, 512:768],
                                      vt[:, 1024:1280])
                        for dc in range(DC):
                            nc.tensor.matmul(v2, lhsT=xi_t[:, dc, qsl],
                                             rhs=wvs["a"][:, dc, :],
                                             start=(dc == 0),
                                             stop=(dc == DC - 1))
                        for dc in range(DC):
                            nc.tensor.matmul(v3, lhsT=xr_t[:, dc, qsl],
                                             rhs=wvs["b"][:, dc, :],
                                             start=(dc == 0),
                                             stop=(dc == DC - 1))
                        for dc in range(DC):
                            nc.tensor.matmul(v1, lhsT=sx_t[:, dc, qsl],
                                             rhs=wvs["r"][:, dc, :],
                                             start=(dc == 0),
                                             stop=(dc == DC - 1))
                        v1s = rt.tile([128, 256], BF16, name="v1s")
                        nc.scalar.copy(v1s[:], v1)
                        vre = v1s[:].rearrange("p (h d) -> p h d", h=HC)
                        v2e = v2.rearrange("p (h d) -> p h d", h=HC)
                        v3e = v3.rearrange("p (h d) -> p h d", h=HC)
                        nc.vector.tensor_sub(vcat[:, tq, :, 0:64], vre, v2e)
                        nc.vector.tensor_add(vcat[:, tq, :, 64:128], vre, v3e)

            # ---------------- attention phase ----------------
            with tc.tile_pool(name="ox", bufs=1) as ox:
                # prefetch o-projection weight combos on the idle sync ring
                ows = {k: ox.tile([128, 2, D], BF16, name=f"ow{k}")
                       for k in ("r", "a", "b")}
                for k in ("r", "a", "b"):
                    nc.sync.dma_start(ows[k][:], ow[k][:])

                with tc.tile_pool(name="att", bufs=3) as att, \
                     tc.tile_pool(name="sm", bufs=2) as sm, \
                     tc.tile_pool(name="attsm", bufs=2) as attsm, \
                     tc.tile_pool(name="sp", bufs=2, space="PSUM") as sp, \
                     tc.tile_pool(name="avp", bufs=2, space="PSUM") as avp:
                    for iw in range(TW):
                        isl = ts(iw, 1024)
                        for h in range(HC):
                            ucc, up0 = h // 2, (h % 2) * 64
                            av = avp.tile([128, 1024], F32, name="av")
                            sA = sm.tile([128, 1024], BF16, name="sA")
                            sB = sm.tile([128, 1024], BF16, name="sB")
                            es_tiles = {}

                            def emit_s(jc, h=h, iw=iw, es_tiles=es_tiles):
                                s = sp.tile([128, 1024], F32, name="s")
                                for half in range(2):
                                    psl = ts(half, 512)
                                    nc.tensor.matmul(
                                        s[:, psl],
                                        lhsT=qkcat[:, HC + h, ts(jc, 128)],
                                        rhs=qkcat[:, h,
                                                  ts(2 * iw + half, 512)],
                                        start=True, stop=True)
                                es = att.tile([128, 1024], BF16, name="es")
                                nc.scalar.activation(es[:], s[:], EXP,
                                                     scale=0.125)
                                es_tiles[jc] = es

                            def emit_av(jc, h=h, av=av, sA=sA, sB=sB,
                                        es_tiles=es_tiles):
                                es = es_tiles.pop(jc)
                                for half in range(2):
                                    psl = ts(half, 512)
                                    nc.tensor.matmul(av[:, psl],
                                                     lhsT=vcat[:, jc, h, :],
                                                     rhs=es[:, psl],
                                                     start=(jc == 0),
                                                     stop=(jc == TQ - 1))
                                if jc == 0:
                                    nc.vector.tensor_copy(sA[:], es[:])
                                elif jc == 1:
                                    nc.gpsimd.tensor_copy(sB[:], es[:])
                                elif jc % 2 == 0:
                                    nc.vector.tensor_add(sA[:], sA[:], es[:])
                                else:
                                    nc.gpsimd.tensor_add(sB[:], sB[:], es[:])

                            emit_s(0)
                            for jc in range(1, TQ):
                                emit_s(jc)
                                emit_av(jc - 1)
                            emit_av(TQ - 1)

                            dnp = sp.tile([128, 1024], F32, name="s")
                            for half in range(2):
                                psl = ts(half, 512)
                                nc.tensor.matmul(dnp[0:1, psl], lhsT=ones[:],
                                                 rhs=sA[:, psl],
                                                 start=True, stop=False)
                                nc.tensor.matmul(dnp[0:1, psl], lhsT=ones[:],
                                                 rhs=sB[:, psl],
                                                 start=False, stop=True)
                            rec = attsm.tile([1, 1024], F32, name="rec")
                            nc.vector.reciprocal_approx_fast(rec[:],
                                                             dnp[0:1, :])
                            bc = attsm.tile([128, 1024], F32, name="bc")
                            nc.gpsimd.partition_broadcast(bc[:], rec[:])
                            nc.vector.tensor_mul(urt[up0:up0 + 64, ucc, isl],
                                                 av[0:64, :], bc[0:64, :])
                            nc.vector.tensor_mul(uit[up0:up0 + 64, ucc, isl],
                                                 av[64:128, :], bc[64:128, :])

                # ---------------- output projection ----------------
                with tc.tile_pool(name="ost", bufs=3) as ost, \
                     tc.tile_pool(name="sup", bufs=1) as sup, \
                     tc.tile_pool(name="op", bufs=2, space="PSUM") as op:
                    su = sup.tile([128, 2, T], BF16, name="su")
                    nc.gpsimd.tensor_add(su[:], urt[:], uit[:])
                    for tq in range(TQ):
                        tslq = ts(tq, 128)
                        for oc in range(2):
                            osl = ts(oc, 512)
                            otr = op.tile([128, 1536], F32, name="ot")
                            o2, o3, o1 = (otr[:, 0:512], otr[:, 512:1024],
                                          otr[:, 1024:1536])
                            for ch in range(2):
                                nc.tensor.matmul(o2, lhsT=uit[:, ch, tslq],
                                                 rhs=ows["a"][:, ch, osl],
                                                 start=(ch == 0),
                                                 stop=(ch == 1))
                            for ch in range(2):
                                nc.tensor.matmul(o3, lhsT=urt[:, ch, tslq],
                                                 rhs=ows["b"][:, ch, osl],
                                                 start=(ch == 0),
                                                 stop=(ch == 1))
                            for ch in range(2):
                                nc.tensor.matmul(o1, lhsT=su[:, ch, tslq],
                                                 rhs=ows["r"][:, ch, osl],
                                                 start=(ch == 0),
                                                 stop=(ch == 1))
                            o1s = ost.tile([128, 512], F32, name="o1s")
                            nc.scalar.copy(o1s[:], o1)
                            str_ = ost.tile([128, 512], F32, name="str")
                            sti = ost.tile([128, 512], F32, name="sti")
                            nc.vector.tensor_sub(str_[:], o1s[:], o2)
                            nc.vector.tensor_add(sti[:], o1s[:], o3)
                            nc.sync.dma_start(outr_d[tslq, osl], str_[:])
                            nc.scalar.dma_start(outi_d[tslq, osl], sti[:])

    nc.compile()
    return nc


def _to_bf16_kxm(arr, parts=128):
    """[K, M] fp32 -> [128, K//128, M] bf16 with K split as (chunk, part)."""
    k, m = arr.shape
    out = arr.reshape(k // parts, parts, m).transpose(1, 0, 2)
    return np.ascontiguousarray(out.astype(ml_dtypes.bfloat16))


def _x_blocks(arr):
    """[T, D] fp32 -> transposed, t5-blocked [128, T5, DC, 512] bf16."""
    xt = arr.T.reshape(DC, 128, T5, 512).transpose(1, 2, 0, 3)
    return np.ascontiguousarray(xt.astype(ml_dtypes.bfloat16))


def _rope_tables():
    inv_freq = 1.0 / (10000.0 ** (np.arange(0, HD, 2, dtype=np.float64) / HD))
    invf64 = np.concatenate([inv_freq, inv_freq])          # [64]
    ang = invf64[:, None] * np.arange(T, dtype=np.float64)[None, :]  # [64, T]
    cos2 = np.tile(np.cos(ang), (2, 1)).astype(ml_dtypes.bfloat16)
    sin2 = np.tile(np.sin(ang), (2, 1)).astype(ml_dtypes.bfloat16)
    return np.ascontiguousarray(cos2), np.ascontiguousarray(sin2)


def kernel(x_real, x_imag, q_wr, q_wi, k_wr, k_wi, v_wr, v_wi, o_wr, o_wi):
    global _COMPILED, LAST_RESULTS
    if _COMPILED is None:
        _COMPILED = _build()
    nc = _COMPILED

    cos2, sin2 = _rope_tables()
    xt = {}
    for b in range(B):
        xt[("r", b)] = _x_blocks(np.asarray(x_real[b], np.float32))
        xt[("i", b)] = _x_blocks(np.asarray(x_imag[b], np.float32))

    in_maps = []
    for core in range(NCORE):
        b, g = core // TP, core % TP
        cols = slice(g * C, (g + 1) * C)
        m = {"xrT": xt[("r", b)], "xiT": xt[("i", b)],
             "cos2": cos2, "sin2": sin2}
        for nm, wr_, wi_ in (("wq", q_wr, q_wi), ("wk", k_wr, k_wi),
                             ("wv", v_wr, v_wi)):
            wr_c = np.asarray(wr_[:, cols], np.float32)
            wi_c = np.asarray(wi_[:, cols], np.float32)
            m[f"{nm}_r"] = _to_bf16_kxm(wr_c)
            m[f"{nm}_a"] = _to_bf16_kxm(wr_c + wi_c)
            m[f"{nm}_b"] = _to_bf16_kxm(wi_c - wr_c)
        owr_c = np.asarray(o_wr[cols, :], np.float32)
        owi_c = np.asarray(o_wi[cols, :], np.float32)
        m["ow_r"] = _to_bf16_kxm(owr_c)
        m["ow_a"] = _to_bf16_kxm(owr_c + owi_c)
        m["ow_b"] = _to_bf16_kxm(owi_c - owr_c)
        in_maps.append(m)

    res = run_bass_kernel_spmd(nc, in_maps, core_ids=list(range(NCORE)))
    LAST_RESULTS = res

    final_r = np.zeros((B, T, D), np.float32)
    final_i = np.zeros((B, T, D), np.float32)
    for core in range(NCORE):
        b = core // TP
        final_r[b] += res.results[core]["out_r"]
        final_i[b] += res.results[core]["out_i"]
    return final_r, final_i


# revision 14
# speedup vs baseline: 1.1798x; 1.1162x over previous
"""ComplexAttentionV3 Trainium2 kernel (v3).

Sharding: 8 cores = data-parallel over batch (2) x tensor-parallel over
heads (16 -> 4 per core). Each core computes q/k/v for its 4 heads
(column-sharded projections), local attention, and a row-sharded
o-projection producing a partial [T, D] output; the host sums the 4
partials per batch.

v3 notes vs v2:
- All four complex projections use the 3-multiplication (Karatsuba)
  form: M1 = (xr+xi)@wr, M2 = xi@(wr+wi), M3 = xr@(wi-wr);
  real = M1-M2, imag = M1+M3. Weight combos precomputed on host;
  xr+xi computed once on gpsimd/vector and shared by q/k/v.
- x arrives in 4 column-blocks of 512 (host layout [128, T5, DC, 512])
  with 2-deep buffer rotation, so the PE starts ~3us in and x DMA
  hides under projection compute.
- Softmax denominators no longer run on the PE: es chunks are
  accumulated on vector (even jc) and gpsimd (odd jc) in bf16, then
  reduced over partitions with 4 small ones-matmuls per (h, iw).
- Attention is software-pipelined (s for jc+1 issued before av for jc,
  av pool double-buffered) so the PE rides through the exp latency.
- o-projection weights prefetch on the idle sync DMA ring at attention
  start; output DMAs alternate between both HWDGE rings.
"""

import numpy as np
import ml_dtypes

import concourse.bacc as bacc
import concourse.tile as tile
from concourse import mybir
from concourse.bass import ts
from concourse.bass_utils import run_bass_kernel_spmd

B, T, D, H = 2, 2048, 1024, 16
HD = 64
NCORE = 8
TP = 4               # head-parallel degree (per batch)
HC = H // TP         # heads per core = 4
C = HC * HD          # local channels = 256
DC = D // 128        # contraction chunks = 8
TQ = T // 128        # 128-row t-chunks = 16
T5 = T // 512        # 512-col t-chunks = 4
TW = T // 1024       # 1024-col t-chunks = 2

F32 = mybir.dt.float32
BF16 = mybir.dt.bfloat16
EXP = mybir.ActivationFunctionType.Exp

LAST_RESULTS = None
_COMPILED = None


def _build():
    nc = bacc.Bacc("TRN2", target_bir_lowering=False, debug=False,
                   num_devices=NCORE)

    def din(name, shape, dt=BF16):
        return nc.dram_tensor(name, shape, dt, kind="ExternalInput").ap()

    xr_d = din("xrT", [128, T5, DC, 512])
    xi_d = din("xiT", [128, T5, DC, 512])
    # Karatsuba combos: r = w_r, a = w_r + w_i, b = w_i - w_r
    wq = {k: din(f"wq_{k}", [128, DC, C]) for k in ("r", "a", "b")}
    wk = {k: din(f"wk_{k}", [128, DC, C]) for k in ("r", "a", "b")}
    wv = {k: din(f"wv_{k}", [128, DC, C]) for k in ("r", "a", "b")}
    ow = {k: din(f"ow_{k}", [128, 2, D]) for k in ("r", "i", "n")}
    cos_d = din("cos2", [128, T])
    sin_d = din("sin2", [128, T])
    outr_d = nc.dram_tensor("out_r", [T, D], F32, kind="ExternalOutput").ap()
    outi_d = nc.dram_tensor("out_i", [T, D], F32, kind="ExternalOutput").ap()

    with tile.TileContext(nc) as tc:
        with tc.tile_pool(name="persist", bufs=1) as persist:
            qkcat = persist.tile([128, 2 * HC, T], BF16, name="qkcat")
            vcat = persist.tile([128, TQ, HC, 128], BF16, name="vcat")
            urt = persist.tile([128, 2, T], BF16, name="urt")
            uit = persist.tile([128, 2, T], BF16, name="uit")
            ones = persist.tile([128, 1], BF16, name="ones")
            nc.vector.memset(ones[:], 1.0)

            # ---------------- projection phase ----------------
            with tc.tile_pool(name="wts", bufs=1) as wts, \
                 tc.tile_pool(name="xblk", bufs=2) as xblk, \
                 tc.tile_pool(name="rt", bufs=2) as rt, \
                 tc.tile_pool(name="pp", bufs=2, space="PSUM") as pp:
                wqs = {k: wts.tile([128, DC, C], BF16, name=f"wq{k}")
                       for k in ("r", "a", "b")}
                wks = {k: wts.tile([128, DC, C], BF16, name=f"wk{k}")
                       for k in ("r", "a", "b")}
                wvs = {k: wts.tile([128, DC, C], BF16, name=f"wv{k}")
                       for k in ("r", "a", "b")}
                cos = wts.tile([128, T], BF16, name="cos")
                sin = wts.tile([128, T], BF16, name="sin")
                # weights + tables on the ACT ring, ordered by first use
                for k in ("a", "b", "r"):
                    nc.scalar.dma_start(wqs[k][:], wq[k][:])
                nc.scalar.dma_start(cos[:], cos_d[:])
                nc.scalar.dma_start(sin[:], sin_d[:])
                for k in ("a", "b", "r"):
                    nc.scalar.dma_start(wks[k][:], wk[k][:])
                for k in ("a", "b", "r"):
                    nc.scalar.dma_start(wvs[k][:], wv[k][:])

                for t5 in range(T5):
                    xi_t = xblk.tile([128, DC, 512], BF16, name="xi")
                    xr_t = xblk.tile([128, DC, 512], BF16, name="xr")
                    sx_t = xblk.tile([128, DC, 512], BF16, name="sx")
                    nc.sync.dma_start(xi_t[:], xi_d[:, t5])
                    nc.sync.dma_start(xr_t[:], xr_d[:, t5])
                    nc.vector.tensor_add(sx_t[:], xr_t[:], xi_t[:])
                    tsl = ts(t5, 512)

                    # q/k transposed [c, t] + RoPE into qkcat
                    for wsrc, hbase in ((wqs, 0), (wks, HC)):
                        for cc in range(2):
                            h0, h1 = hbase + 2 * cc, hbase + 2 * cc + 1
                            csl = ts(cc, 128)
                            trio = pp.tile([128, 1536], F32, name="trio")
                            m2, m3, m1 = (trio[:, 0:512], trio[:, 512:1024],
                                          trio[:, 1024:1536])
                            for dc in range(DC):
                                nc.tensor.matmul(
                                    m2, lhsT=wsrc["a"][:, dc, csl],
                                    rhs=xi_t[:, dc, :],
                                    start=(dc == 0), stop=(dc == DC - 1))
                            for dc in range(DC):
                                nc.tensor.matmul(
                                    m3, lhsT=wsrc["b"][:, dc, csl],
                                    rhs=xr_t[:, dc, :],
                                    start=(dc == 0), stop=(dc == DC - 1))
                            for dc in range(DC):
                                nc.tensor.matmul(
                                    m1, lhsT=wsrc["r"][:, dc, csl],
                                    rhs=sx_t[:, dc, :],
                                    start=(dc == 0), stop=(dc == DC - 1))
                            m1s = rt.tile([128, 512], BF16, name="m1s")
                            nc.scalar.copy(m1s[:], m1)
                            tr = rt.tile([128, 512], BF16, name="tr")
                            ti = rt.tile([128, 512], BF16, name="ti")
                            nc.vector.tensor_sub(tr[:], m1s[:], m2)
                            nc.vector.tensor_add(ti[:], m1s[:], m3)
                            t1 = rt.tile([128, 512], BF16, name="t1")
                            t2 = rt.tile([128, 512], BF16, name="t2")
                            t3 = rt.tile([128, 512], BF16, name="t3")
                            t4 = rt.tile([128, 512], BF16, name="t4")
                            nc.vector.tensor_mul(t1[:], tr[:], cos[:, tsl])
                            nc.vector.tensor_mul(t2[:], ti[:], sin[:, tsl])
                            nc.vector.tensor_mul(t3[:], tr[:], sin[:, tsl])
                            nc.vector.tensor_mul(t4[:], ti[:], cos[:, tsl])
                            nc.vector.tensor_sub(qkcat[0:64, h0, tsl],
                                                 t1[0:64, :], t2[0:64, :])
                            nc.vector.tensor_sub(qkcat[0:64, h1, tsl],
                                                 t1[64:128, :], t2[64:128, :])
                            nc.vector.tensor_add(qkcat[64:128, h0, tsl],
                                                 t3[0:64, :], t4[0:64, :])
                            nc.vector.tensor_add(qkcat[64:128, h1, tsl],
                                                 t3[64:128, :], t4[64:128, :])

                    # v natural [t, c] for the 4 tq chunks of this t5
                    for tql in range(4):
                        tq = 4 * t5 + tql
                        qsl = ts(tql, 128)
                        vt = pp.tile([128, 1536], F32, name="trio")
                        v2, v3, v1 = (vt[:, 0:256], vt[:, 512:768],
                                      vt[:, 1024:1280])
                        for dc in range(DC):
                            nc.tensor.matmul(v2, lhsT=xi_t[:, dc, qsl],
                                             rhs=wvs["a"][:, dc, :],
                                             start=(dc == 0),
                                             stop=(dc == DC - 1))
                        for dc in range(DC):
                            nc.tensor.matmul(v3, lhsT=xr_t[:, dc, qsl],
                                             rhs=wvs["b"][:, dc, :],
                                             start=(dc == 0),
                                             stop=(dc == DC - 1))
                        for dc in range(DC):
                            nc.tensor.matmul(v1, lhsT=sx_t[:, dc, qsl],
                                             rhs=wvs["r"][:, dc, :],
                                             start=(dc == 0),
                                             stop=(dc == DC - 1))
                        v1s = rt.tile([128, 256], BF16, name="v1s")
                        nc.scalar.copy(v1s[:], v1)
                        vre = v1s[:].rearrange("p (h d) -> p h d", h=HC)
                        v2e = v2.rearrange("p (h d) -> p h d", h=HC)
                        v3e = v3.rearrange("p (h d) -> p h d", h=HC)
                        nc.vector.tensor_sub(vcat[:, tq, :, 0:64], vre, v2e)
                        nc.vector.tensor_add(vcat[:, tq, :, 64:128], vre, v3e)

            # ---------------- attention phase ----------------
            with tc.tile_pool(name="ox", bufs=1) as ox:
                # prefetch o-projection weight combos on the idle sync ring
                ows = {k: ox.tile([128, 2, D], BF16, name=f"ow{k}")
                       for k in ("r", "i", "n")}
                for k in ("r", "i", "n"):
                    nc.sync.dma_start(ows[k][:], ow[k][:])

                with tc.tile_pool(name="att", bufs=6) as att, \
                     tc.tile_pool(name="sm", bufs=2) as sm, \
                     tc.tile_pool(name="attsm", bufs=2) as attsm, \
                     tc.tile_pool(name="sp", bufs=2, space="PSUM") as sp, \
                     tc.tile_pool(name="avp", bufs=2, space="PSUM") as avp:
                    from concourse import bass_isa
                    slots = [(iw, h) for iw in range(TW) for h in range(HC)]
                    es_tiles = {}
                    av_tiles = {}
                    sA_tiles = {}

                    def emit_s(si, jc):
                        iw, h = slots[si]
                        s = sp.tile([128, 1024], F32, name="s")
                        for half in range(2):
                            psl = ts(half, 512)
                            nc.tensor.matmul(
                                s[:, psl],
                                lhsT=qkcat[:, HC + h, ts(jc, 128)],
                                rhs=qkcat[:, h, ts(2 * iw + half, 512)],
                                start=True, stop=True)
                        es = att.tile([128, 1024], BF16, name="es")
                        nc.scalar.activation(es[:], s[:], EXP, scale=0.125)
                        es_tiles[(si, jc)] = es

                    def emit_av(si, jc):
                        iw, h = slots[si]
                        es = es_tiles.pop((si, jc))
                        if jc == 0:
                            av_tiles[si] = avp.tile([128, 1024], F32,
                                                    name="av")
                        av = av_tiles[si]
                        for half in range(2):
                            psl = ts(half, 512)
                            nc.tensor.matmul(av[:, psl],
                                             lhsT=vcat[:, jc, h, :],
                                             rhs=es[:, psl],
                                             start=(jc == 0),
                                             stop=(jc == TQ - 1))
                        if jc == 0:
                            sA_tiles[si] = sm.tile([128, 1024], BF16,
                                                   name="sA")
                            nc.vector.tensor_copy(sA_tiles[si][:], es[:])
                        else:
                            sA = sA_tiles[si]
                            nc.vector.tensor_add(sA[:], sA[:], es[:])
                        if jc == TQ - 1:
                            finish_slot(si)

                    def finish_slot(si):
                        iw, h = slots[si]
                        isl = ts(iw, 1024)
                        ucc, up0 = h // 2, (h % 2) * 64
                        av = av_tiles.pop(si)
                        sA = sA_tiles.pop(si)
                        dnb = attsm.tile([128, 1024], F32, name="dnb")
                        nc.gpsimd.partition_all_reduce(
                            dnb[:], sA[:], channels=128,
                            reduce_op=bass_isa.ReduceOp.add)
                        rec = attsm.tile([128, 1024], F32, name="rec")
                        nc.vector.reciprocal_approx_fast(rec[:], dnb[:])
                        nc.vector.tensor_mul(urt[up0:up0 + 64, ucc, isl],
                                             av[0:64, :], rec[0:64, :])
                        nc.vector.tensor_mul(uit[up0:up0 + 64, ucc, isl],
                                             av[64:128, :], rec[64:128, :])

                    LAG = 3
                    stream = [(si, jc) for si in range(len(slots))
                              for jc in range(TQ)]
                    for pos, (si, jc) in enumerate(stream):
                        emit_s(si, jc)
                        if pos >= LAG:
                            emit_av(*stream[pos - LAG])
                    for pos in range(len(stream) - LAG, len(stream)):
                        emit_av(*stream[pos])

                # ---------------- output projection ----------------
                with tc.tile_pool(name="ost", bufs=3) as ost, \
                     tc.tile_pool(name="op", bufs=2, space="PSUM") as op:
                    for tq in range(TQ):
                        tslq = ts(tq, 128)
                        for oc in range(2):
                            osl = ts(oc, 512)
                            pr = op.tile([128, 512], F32, name="pr")
                            pi = op.tile([128, 512], F32, name="pi")
                            nc.tensor.matmul(pr[:], lhsT=urt[:, 0, tslq],
                                             rhs=ows["r"][:, 0, osl],
                                             start=True, stop=False)
                            nc.tensor.matmul(pr[:], lhsT=urt[:, 1, tslq],
                                             rhs=ows["r"][:, 1, osl],
                                             start=False, stop=False)
                            nc.tensor.matmul(pr[:], lhsT=uit[:, 0, tslq],
                                             rhs=ows["n"][:, 0, osl],
                                             start=False, stop=False)
                            nc.tensor.matmul(pr[:], lhsT=uit[:, 1, tslq],
                                             rhs=ows["n"][:, 1, osl],
                                             start=False, stop=True)
                            nc.tensor.matmul(pi[:], lhsT=urt[:, 0, tslq],
                                             rhs=ows["i"][:, 0, osl],
                                             start=True, stop=False)
                            nc.tensor.matmul(pi[:], lhsT=urt[:, 1, tslq],
                                             rhs=ows["i"][:, 1, osl],
                                             start=False, stop=False)
                            nc.tensor.matmul(pi[:], lhsT=uit[:, 0, tslq],
                                             rhs=ows["r"][:, 0, osl],
                                             start=False, stop=False)
                            nc.tensor.matmul(pi[:], lhsT=uit[:, 1, tslq],
                                             rhs=ows["r"][:, 1, osl],
                                             start=False, stop=True)
                            str_ = ost.tile([128, 512], F32, name="str")
                            sti = ost.tile([128, 512], F32, name="sti")
                            nc.scalar.copy(str_[:], pr[:])
                            nc.vector.tensor_copy(sti[:], pi[:])
                            nc.sync.dma_start(outr_d[tslq, osl], str_[:])
                            nc.scalar.dma_start(outi_d[tslq, osl], sti[:])

    nc.compile()
    return nc


def _to_bf16_kxm(arr, parts=128):
    """[K, M] fp32 -> [128, K//128, M] bf16 with K split as (chunk, part)."""
    k, m = arr.shape
    out = arr.reshape(k // parts, parts, m).transpose(1, 0, 2)
    return np.ascontiguousarray(out.astype(ml_dtypes.bfloat16))


def _x_blocks(arr):
    """[T, D] fp32 -> transposed, t5-blocked [128, T5, DC, 512] bf16."""
    xt = arr.T.reshape(DC, 128, T5, 512).transpose(1, 2, 0, 3)
    return np.ascontiguousarray(xt.astype(ml_dtypes.bfloat16))


def _rope_tables():
    inv_freq = 1.0 / (10000.0 ** (np.arange(0, HD, 2, dtype=np.float64) / HD))
    invf64 = np.concatenate([inv_freq, inv_freq])          # [64]
    ang = invf64[:, None] * np.arange(T, dtype=np.float64)[None, :]  # [64, T]
    cos2 = np.tile(np.cos(ang), (2, 1)).astype(ml_dtypes.bfloat16)
    sin2 = np.tile(np.sin(ang), (2, 1)).astype(ml_dtypes.bfloat16)
    return np.ascontiguousarray(cos2), np.ascontiguousarray(sin2)


def kernel(x_real, x_imag, q_wr, q_wi, k_wr, k_wi, v_wr, v_wi, o_wr, o_wi):
    global _COMPILED, LAST_RESULTS
    if _COMPILED is None:
        _COMPILED = _build()
    nc = _COMPILED

    cos2, sin2 = _rope_tables()
    xt = {}
    for b in range(B):
        xt[("r", b)] = _x_blocks(np.asarray(x_real[b], np.float32))
        xt[("i", b)] = _x_blocks(np.asarray(x_imag[b], np.float32))

    in_maps = []
    for core in range(NCORE):
        b, g = core // TP, core % TP
        cols = slice(g * C, (g + 1) * C)
        m = {"xrT": xt[("r", b)], "xiT": xt[("i", b)],
             "cos2": cos2, "sin2": sin2}
        for nm, wr_, wi_ in (("wq", q_wr, q_wi), ("wk", k_wr, k_wi),
                             ("wv", v_wr, v_wi)):
            wr_c = np.asarray(wr_[:, cols], np.float32)
            wi_c = np.asarray(wi_[:, cols], np.float32)
            m[f"{nm}_r"] = _to_bf16_kxm(wr_c)
            m[f"{nm}_a"] = _to_bf16_kxm(wr_c + wi_c)
            m[f"{nm}_b"] = _to_bf16_kxm(wi_c - wr_c)
        owr_c = np.asarray(o_wr[cols, :], np.float32)
        owi_c = np.asarray(o_wi[cols, :], np.float32)
        m["ow_r"] = _to_bf16_kxm(owr_c)
        m["ow_i"] = _to_bf16_kxm(owi_c)
        m["ow_n"] = _to_bf16_kxm(-owi_c)
        in_maps.append(m)

    res = run_bass_kernel_spmd(nc, in_maps, core_ids=list(range(NCORE)))
    LAST_RESULTS = res

    final_r = np.zeros((B, T, D), np.float32)
    final_i = np.zeros((B, T, D), np.float32)
    for core in range(NCORE):
        b = core // TP
        final_r[b] += res.results[core]["out_r"]
        final_i[b] += res.results[core]["out_i"]
    return final_r, final_i


# revision 15
# speedup vs baseline: 1.4036x; 1.1897x over previous
"""ComplexAttentionV3 Trainium2 kernel (v3).

Sharding: 8 cores = data-parallel over batch (2) x tensor-parallel over
heads (16 -> 4 per core). Each core computes q/k/v for its 4 heads
(column-sharded projections), local attention, and a row-sharded
o-projection producing a partial [T, D] output; the host sums the 4
partials per batch.

v3 notes vs v2:
- All four complex projections use the 3-multiplication (Karatsuba)
  form: M1 = (xr+xi)@wr, M2 = xi@(wr+wi), M3 = xr@(wi-wr);
  real = M1-M2, imag = M1+M3. Weight combos precomputed on host;
  xr+xi computed once on gpsimd/vector and shared by q/k/v.
- x arrives in 4 column-blocks of 512 (host layout [128, T5, DC, 512])
  with 2-deep buffer rotation, so the PE starts ~3us in and x DMA
  hides under projection compute.
- Softmax denominators no longer run on the PE: es chunks are
  accumulated on vector (even jc) and gpsimd (odd jc) in bf16, then
  reduced over partitions with 4 small ones-matmuls per (h, iw).
- Attention is software-pipelined (s for jc+1 issued before av for jc,
  av pool double-buffered) so the PE rides through the exp latency.
- o-projection weights prefetch on the idle sync DMA ring at attention
  start; output DMAs alternate between both HWDGE rings.
"""

import numpy as np
import ml_dtypes

import concourse.bacc as bacc
import concourse.tile as tile
from concourse import mybir
from concourse.bass import ts
from concourse.bass_utils import run_bass_kernel_spmd

B, T, D, H = 2, 2048, 1024, 16
HD = 64
NCORE = 8
TP = 4               # head-parallel degree (per batch)
HC = H // TP         # heads per core = 4
C = HC * HD          # local channels = 256
DC = D // 128        # contraction chunks = 8
TQ = T // 128        # 128-row t-chunks = 16
T5 = T // 512        # 512-col t-chunks = 4
TW = T // 1024       # 1024-col t-chunks = 2

F32 = mybir.dt.float32
BF16 = mybir.dt.bfloat16
EXP = mybir.ActivationFunctionType.Exp

LAST_RESULTS = None
_COMPILED = None


def _build():
    nc = bacc.Bacc("TRN2", target_bir_lowering=False, debug=False,
                   num_devices=NCORE)

    def din(name, shape, dt=BF16):
        return nc.dram_tensor(name, shape, dt, kind="ExternalInput").ap()

    xr_d = din("xrT", [128, T5, DC, 512])
    xi_d = din("xiT", [128, T5, DC, 512])
    # Karatsuba combos: r = w_r, a = w_r + w_i, b = w_i - w_r
    wq = {k: din(f"wq_{k}", [128, DC, C]) for k in ("r", "a", "b")}
    wk = {k: din(f"wk_{k}", [128, DC, C]) for k in ("r", "a", "b")}
    wv = {k: din(f"wv_{k}", [128, DC, C]) for k in ("r", "a", "b")}
    ow = {k: din(f"ow_{k}", [128, 2, D]) for k in ("r", "i", "n")}
    cos_d = din("cos2", [128, T])
    sin_d = din("sin2", [128, T])
    outr_d = nc.dram_tensor("out_r", [T, D], F32, kind="ExternalOutput").ap()
    outi_d = nc.dram_tensor("out_i", [T, D], F32, kind="ExternalOutput").ap()

    with tile.TileContext(nc) as tc:
        with tc.tile_pool(name="persist", bufs=1) as persist:
            qkcat = persist.tile([128, 2 * HC, T], BF16, name="qkcat")
            vcat = persist.tile([128, TQ, HC, 128], BF16, name="vcat")
            urt = persist.tile([128, 2, T], BF16, name="urt")
            uit = persist.tile([128, 2, T], BF16, name="uit")
            ones = persist.tile([128, 1], BF16, name="ones")
            nc.vector.memset(ones[:], 1.0)

            # ---------------- projection phase ----------------
            with tc.tile_pool(name="wts", bufs=1) as wts, \
                 tc.tile_pool(name="xblk", bufs=2) as xblk, \
                 tc.tile_pool(name="rt", bufs=2) as rt, \
                 tc.tile_pool(name="pp", bufs=2, space="PSUM") as pp:
                wqs = {k: wts.tile([128, DC, C], BF16, name=f"wq{k}")
                       for k in ("r", "a", "b")}
                wks = {k: wts.tile([128, DC, C], BF16, name=f"wk{k}")
                       for k in ("r", "a", "b")}
                wvs = {k: wts.tile([128, DC, C], BF16, name=f"wv{k}")
                       for k in ("r", "a", "b")}
                cos = wts.tile([128, T], BF16, name="cos")
                sin = wts.tile([128, T], BF16, name="sin")
                # weights + tables on the ACT ring, ordered by first use
                for k in ("a", "b", "r"):
                    nc.scalar.dma_start(wqs[k][:], wq[k][:])
                nc.scalar.dma_start(cos[:], cos_d[:])
                nc.scalar.dma_start(sin[:], sin_d[:])
                for k in ("a", "b", "r"):
                    nc.scalar.dma_start(wks[k][:], wk[k][:])
                for k in ("a", "b", "r"):
                    nc.scalar.dma_start(wvs[k][:], wv[k][:])

                for t5 in range(T5):
                    xi_t = xblk.tile([128, DC, 512], BF16, name="xi")
                    xr_t = xblk.tile([128, DC, 512], BF16, name="xr")
                    sx_t = xblk.tile([128, DC, 512], BF16, name="sx")
                    nc.sync.dma_start(xi_t[:], xi_d[:, t5])
                    nc.sync.dma_start(xr_t[:], xr_d[:, t5])
                    nc.vector.tensor_add(sx_t[:], xr_t[:], xi_t[:])
                    tsl = ts(t5, 512)

                    # q/k transposed [c, t] + RoPE into qkcat
                    for wsrc, hbase in ((wqs, 0), (wks, HC)):
                        for cc in range(2):
                            h0, h1 = hbase + 2 * cc, hbase + 2 * cc + 1
                            csl = ts(cc, 128)
                            trio = pp.tile([128, 1536], F32, name="trio")
                            m2, m3, m1 = (trio[:, 0:512], trio[:, 512:1024],
                                          trio[:, 1024:1536])
                            for dc in range(DC):
                                nc.tensor.matmul(
                                    m2, lhsT=wsrc["a"][:, dc, csl],
                                    rhs=xi_t[:, dc, :],
                                    start=(dc == 0), stop=(dc == DC - 1))
                            for dc in range(DC):
                                nc.tensor.matmul(
                                    m3, lhsT=wsrc["b"][:, dc, csl],
                                    rhs=xr_t[:, dc, :],
                                    start=(dc == 0), stop=(dc == DC - 1))
                            for dc in range(DC):
                                nc.tensor.matmul(
                                    m1, lhsT=wsrc["r"][:, dc, csl],
                                    rhs=sx_t[:, dc, :],
                                    start=(dc == 0), stop=(dc == DC - 1))
                            m1s = rt.tile([128, 512], BF16, name="m1s")
                            nc.scalar.copy(m1s[:], m1)
                            tr = rt.tile([128, 512], BF16, name="tr")
                            ti = rt.tile([128, 512], BF16, name="ti")
                            nc.vector.tensor_sub(tr[:], m1s[:], m2)
                            nc.vector.tensor_add(ti[:], m1s[:], m3)
                            t1 = rt.tile([128, 512], BF16, name="t1")
                            t2 = rt.tile([128, 512], BF16, name="t2")
                            t3 = rt.tile([128, 512], BF16, name="t3")
                            t4 = rt.tile([128, 512], BF16, name="t4")
                            nc.vector.tensor_mul(t1[:], tr[:], cos[:, tsl])
                            nc.vector.tensor_mul(t2[:], ti[:], sin[:, tsl])
                            nc.vector.tensor_mul(t3[:], tr[:], sin[:, tsl])
                            nc.vector.tensor_mul(t4[:], ti[:], cos[:, tsl])
                            nc.vector.tensor_sub(qkcat[0:64, h0, tsl],
                                                 t1[0:64, :], t2[0:64, :])
                            nc.vector.tensor_sub(qkcat[0:64, h1, tsl],
                                                 t1[64:128, :], t2[64:128, :])
                            nc.vector.tensor_add(qkcat[64:128, h0, tsl],
                                                 t3[0:64, :], t4[0:64, :])
                            nc.vector.tensor_add(qkcat[64:128, h1, tsl],
                                                 t3[64:128, :], t4[64:128, :])

                    # v natural [t, c] for the 4 tq chunks of this t5
                    for tql in range(4):
                        tq = 4 * t5 + tql
                        qsl = ts(tql, 128)
                        vt = pp.tile([128, 1536], F32, name="trio")
                        v2, v3, v1 = (vt[:, 0:256], vt[:, 512:768],
                                      vt[:, 1024:1280])
                        for dc in range(DC):
                            nc.tensor.matmul(v2, lhsT=xi_t[:, dc, qsl],
                                             rhs=wvs["a"][:, dc, :],
                                             start=(dc == 0),
                                             stop=(dc == DC - 1))
                        for dc in range(DC):
                            nc.tensor.matmul(v3, lhsT=xr_t[:, dc, qsl],
                                             rhs=wvs["b"][:, dc, :],
                                             start=(dc == 0),
                                             stop=(dc == DC - 1))
                        for dc in range(DC):
                            nc.tensor.matmul(v1, lhsT=sx_t[:, dc, qsl],
                                             rhs=wvs["r"][:, dc, :],
                                             start=(dc == 0),
                                             stop=(dc == DC - 1))
                        v1s = rt.tile([128, 256], BF16, name="v1s")
                        nc.scalar.copy(v1s[:], v1)
                        vre = v1s[:].rearrange("p (h d) -> p h d", h=HC)
                        v2e = v2.rearrange("p (h d) -> p h d", h=HC)
                        v3e = v3.rearrange("p (h d) -> p h d", h=HC)
                        nc.vector.tensor_sub(vcat[:, tq, :, 0:64], vre, v2e)
                        nc.vector.tensor_add(vcat[:, tq, :, 64:128], vre, v3e)

            # ---------------- attention phase ----------------
            with tc.tile_pool(name="ox", bufs=1) as ox:
                # prefetch o-projection weight combos on the idle sync ring
                ows = {k: ox.tile([128, 2, D], BF16, name=f"ow{k}")
                       for k in ("r", "i", "n")}
                for k in ("r", "i", "n"):
                    nc.sync.dma_start(ows[k][:], ow[k][:])

                with tc.tile_pool(name="att", bufs=6) as att, \
                     tc.tile_pool(name="sm", bufs=2) as sm, \
                     tc.tile_pool(name="attsm", bufs=2) as attsm, \
                     tc.tile_pool(name="sp", bufs=2, space="PSUM") as sp, \
                     tc.tile_pool(name="avp", bufs=2, space="PSUM") as avp:
                    from concourse import bass_isa
                    slots = [(iw, h) for iw in range(TW) for h in range(HC)]
                    es_tiles = {}
                    av_tiles = {}
                    sA_tiles = {}

                    def emit_s(si, jc):
                        iw, h = slots[si]
                        s = sp.tile([128, 1024], F32, name="s")
                        for half in range(2):
                            psl = ts(half, 512)
                            nc.tensor.matmul(
                                s[:, psl],
                                lhsT=qkcat[:, HC + h, ts(jc, 128)],
                                rhs=qkcat[:, h, ts(2 * iw + half, 512)],
                                start=True, stop=True)
                        es = att.tile([128, 1024], BF16, name="es")
                        nc.scalar.activation(es[:], s[:], EXP, scale=0.125)
                        es_tiles[(si, jc)] = es

                    pending = {}

                    def emit_av(si, jc):
                        iw, h = slots[si]
                        es = es_tiles.pop((si, jc))
                        if jc == 0:
                            av_tiles[si] = avp.tile([128, 1024], F32,
                                                    name="av")
                        av = av_tiles[si]
                        for half in range(2):
                            psl = ts(half, 512)
                            nc.tensor.matmul(av[:, psl],
                                             lhsT=vcat[:, jc, h, :],
                                             rhs=es[:, psl],
                                             start=(jc == 0),
                                             stop=(jc == TQ - 1))
                        if jc == 0:
                            sA_tiles[si] = sm.tile([128, 1024], BF16,
                                                   name="sA")
                            nc.vector.tensor_copy(sA_tiles[si][:], es[:])
                        else:
                            sA = sA_tiles[si]
                            nc.vector.tensor_add(sA[:], sA[:], es[:])
                        if jc == 4 and (si - 1) in pending:
                            finish_slot(si - 1)
                        if jc == TQ - 1:
                            # launch the (slow) partition reduction now; the
                            # reciprocal + normalize run mid-next-head so the
                            # vector queue never blocks on gpsimd latency
                            sA = sA_tiles.pop(si)
                            dnb = attsm.tile([128, 1024], F32, name="dnb")
                            nc.gpsimd.partition_all_reduce(
                                dnb[:], sA[:], channels=128,
                                reduce_op=bass_isa.ReduceOp.add)
                            pending[si] = dnb

                    def finish_slot(si):
                        iw, h = slots[si]
                        isl = ts(iw, 1024)
                        ucc, up0 = h // 2, (h % 2) * 64
                        av = av_tiles.pop(si)
                        dnb = pending.pop(si)
                        rec = attsm.tile([128, 1024], F32, name="rec")
                        nc.vector.reciprocal_approx_fast(rec[:], dnb[:])
                        nc.vector.tensor_mul(urt[up0:up0 + 64, ucc, isl],
                                             av[0:64, :], rec[0:64, :])
                        nc.vector.tensor_mul(uit[up0:up0 + 64, ucc, isl],
                                             av[64:128, :], rec[64:128, :])

                    LAG = 3
                    stream = [(si, jc) for si in range(len(slots))
                              for jc in range(TQ)]
                    for pos, (si, jc) in enumerate(stream):
                        emit_s(si, jc)
                        if pos >= LAG:
                            emit_av(*stream[pos - LAG])
                    for pos in range(len(stream) - LAG, len(stream)):
                        emit_av(*stream[pos])
                    finish_slot(len(slots) - 1)

                # ---------------- output projection ----------------
                with tc.tile_pool(name="ost", bufs=3) as ost, \
                     tc.tile_pool(name="op", bufs=2, space="PSUM") as op:
                    for tq in range(TQ):
                        tslq = ts(tq, 128)
                        for oc in range(2):
                            osl = ts(oc, 512)
                            pr = op.tile([128, 512], F32, name="pr")
                            pi = op.tile([128, 512], F32, name="pi")
                            nc.tensor.matmul(pr[:], lhsT=urt[:, 0, tslq],
                                             rhs=ows["r"][:, 0, osl],
                                             start=True, stop=False)
                            nc.tensor.matmul(pr[:], lhsT=urt[:, 1, tslq],
                                             rhs=ows["r"][:, 1, osl],
                                             start=False, stop=False)
                            nc.tensor.matmul(pr[:], lhsT=uit[:, 0, tslq],
                                             rhs=ows["n"][:, 0, osl],
                                             start=False, stop=False)
                            nc.tensor.matmul(pr[:], lhsT=uit[:, 1, tslq],
                                             rhs=ows["n"][:, 1, osl],
                                             start=False, stop=True)
                            nc.tensor.matmul(pi[:], lhsT=urt[:, 0, tslq],
                                             rhs=ows["i"][:, 0, osl],
                                             start=True, stop=False)
                            nc.tensor.matmul(pi[:], lhsT=urt[:, 1, tslq],
                                             rhs=ows["i"][:, 1, osl],
                                             start=False, stop=False)
                            nc.tensor.matmul(pi[:], lhsT=uit[:, 0, tslq],
                                             rhs=ows["r"][:, 0, osl],
                                             start=False, stop=False)
                            nc.tensor.matmul(pi[:], lhsT=uit[:, 1, tslq],
                                             rhs=ows["r"][:, 1, osl],
                                             start=False, stop=True)
                            str_ = ost.tile([128, 512], F32, name="str")
                            sti = ost.tile([128, 512], F32, name="sti")
                            nc.scalar.copy(str_[:], pr[:])
                            nc.vector.tensor_copy(sti[:], pi[:])
                            nc.sync.dma_start(outr_d[tslq, osl], str_[:])
                            nc.scalar.dma_start(outi_d[tslq, osl], sti[:])

    nc.compile()
    return nc


def _to_bf16_kxm(arr, parts=128):
    """[K, M] fp32 -> [128, K//128, M] bf16 with K split as (chunk, part)."""
    k, m = arr.shape
    out = arr.reshape(k // parts, parts, m).transpose(1, 0, 2)
    return np.ascontiguousarray(out.astype(ml_dtypes.bfloat16))


def _x_blocks(arr):
    """[T, D] fp32 -> transposed, t5-blocked [128, T5, DC, 512] bf16."""
    xt = arr.T.reshape(DC, 128, T5, 512).transpose(1, 2, 0, 3)
    return np.ascontiguousarray(xt.astype(ml_dtypes.bfloat16))


def _rope_tables():
    inv_freq = 1.0 / (10000.0 ** (np.arange(0, HD, 2, dtype=np.float64) / HD))
    invf64 = np.concatenate([inv_freq, inv_freq])          # [64]
    ang = invf64[:, None] * np.arange(T, dtype=np.float64)[None, :]  # [64, T]
    cos2 = np.tile(np.cos(ang), (2, 1)).astype(ml_dtypes.bfloat16)
    sin2 = np.tile(np.sin(ang), (2, 1)).astype(ml_dtypes.bfloat16)
    return np.ascontiguousarray(cos2), np.ascontiguousarray(sin2)


def kernel(x_real, x_imag, q_wr, q_wi, k_wr, k_wi, v_wr, v_wi, o_wr, o_wi):
    global _COMPILED, LAST_RESULTS
    if _COMPILED is None:
        _COMPILED = _build()
    nc = _COMPILED

    cos2, sin2 = _rope_tables()
    xt = {}
    for b in range(B):
        xt[("r", b)] = _x_blocks(np.asarray(x_real[b], np.float32))
        xt[("i", b)] = _x_blocks(np.asarray(x_imag[b], np.float32))

    in_maps = []
    for core in range(NCORE):
        b, g = core // TP, core % TP
        cols = slice(g * C, (g + 1) * C)
        m = {"xrT": xt[("r", b)], "xiT": xt[("i", b)],
             "cos2": cos2, "sin2": sin2}
        for nm, wr_, wi_ in (("wq", q_wr, q_wi), ("wk", k_wr, k_wi),
                             ("wv", v_wr, v_wi)):
            wr_c = np.asarray(wr_[:, cols], np.float32)
            wi_c = np.asarray(wi_[:, cols], np.float32)
            m[f"{nm}_r"] = _to_bf16_kxm(wr_c)
            m[f"{nm}_a"] = _to_bf16_kxm(wr_c + wi_c)
            m[f"{nm}_b"] = _to_bf16_kxm(wi_c - wr_c)
        owr_c = np.asarray(o_wr[cols, :], np.float32)
        owi_c = np.asarray(o_wi[cols, :], np.float32)
        m["ow_r"] = _to_bf16_kxm(owr_c)
        m["ow_i"] = _to_bf16_kxm(owi_c)
        m["ow_n"] = _to_bf16_kxm(-owi_c)
        in_maps.append(m)

    res = run_bass_kernel_spmd(nc, in_maps, core_ids=list(range(NCORE)))
    LAST_RESULTS = res

    final_r = np.zeros((B, T, D), np.float32)
    final_i = np.zeros((B, T, D), np.float32)
    for core in range(NCORE):
        b = core // TP
        final_r[b] += res.results[core]["out_r"]
        final_i[b] += res.results[core]["out_i"]
    return final_r, final_i


# revision 18
# speedup vs baseline: 1.4097x; 1.0044x over previous
"""ComplexAttentionV3 Trainium2 kernel (v3).

Sharding: 8 cores = data-parallel over batch (2) x tensor-parallel over
heads (16 -> 4 per core). Each core computes q/k/v for its 4 heads
(column-sharded projections), local attention, and a row-sharded
o-projection producing a partial [T, D] output; the host sums the 4
partials per batch.

v3 notes vs v2:
- All four complex projections use the 3-multiplication (Karatsuba)
  form: M1 = (xr+xi)@wr, M2 = xi@(wr+wi), M3 = xr@(wi-wr);
  real = M1-M2, imag = M1+M3. Weight combos precomputed on host;
  xr+xi computed once on gpsimd/vector and shared by q/k/v.
- x arrives in 4 column-blocks of 512 (host layout [128, T5, DC, 512])
  with 2-deep buffer rotation, so the PE starts ~3us in and x DMA
  hides under projection compute.
- Softmax denominators no longer run on the PE: es chunks are
  accumulated on vector (even jc) and gpsimd (odd jc) in bf16, then
  reduced over partitions with 4 small ones-matmuls per (h, iw).
- Attention is software-pipelined (s for jc+1 issued before av for jc,
  av pool double-buffered) so the PE rides through the exp latency.
- o-projection weights prefetch on the idle sync DMA ring at attention
  start; output DMAs alternate between both HWDGE rings.
"""

import numpy as np
import ml_dtypes

import concourse.bacc as bacc
import concourse.tile as tile
from concourse import mybir
from concourse.bass import ts
from concourse.bass_utils import run_bass_kernel_spmd

B, T, D, H = 2, 2048, 1024, 16
HD = 64
NCORE = 8
TP = 4               # head-parallel degree (per batch)
HC = H // TP         # heads per core = 4
C = HC * HD          # local channels = 256
DC = D // 128        # contraction chunks = 8
TQ = T // 128        # 128-row t-chunks = 16
T5 = T // 512        # 512-col t-chunks = 4
TW = T // 1024       # 1024-col t-chunks = 2

F32 = mybir.dt.float32
BF16 = mybir.dt.bfloat16
EXP = mybir.ActivationFunctionType.Exp

LAST_RESULTS = None
_COMPILED = None


def _build():
    nc = bacc.Bacc("TRN2", target_bir_lowering=False, debug=False,
                   num_devices=NCORE)

    def din(name, shape, dt=BF16):
        return nc.dram_tensor(name, shape, dt, kind="ExternalInput").ap()

    xr_d = din("xrT", [128, T5, DC, 512])
    xi_d = din("xiT", [128, T5, DC, 512])
    # Karatsuba combos: r = w_r, a = w_r + w_i, b = w_i - w_r
    wq = {k: din(f"wq_{k}", [128, DC, C]) for k in ("r", "a", "b")}
    wk = {k: din(f"wk_{k}", [128, DC, C]) for k in ("r", "a", "b")}
    wv = {k: din(f"wv_{k}", [128, DC, C]) for k in ("r", "a", "b")}
    ow = {k: din(f"ow_{k}", [128, 2, D]) for k in ("r", "i", "n")}
    cos_d = din("cos2", [128, T])
    sin_d = din("sin2", [128, T])
    outr_d = nc.dram_tensor("out_r", [T, D], F32, kind="ExternalOutput").ap()
    outi_d = nc.dram_tensor("out_i", [T, D], F32, kind="ExternalOutput").ap()

    with tile.TileContext(nc) as tc:
        with tc.tile_pool(name="persist", bufs=1) as persist:
            qkcat = persist.tile([128, 2 * HC, T], BF16, name="qkcat")
            vcat = persist.tile([128, TQ, HC, 128], BF16, name="vcat")
            urt = persist.tile([128, 2, T], BF16, name="urt")
            uit = persist.tile([128, 2, T], BF16, name="uit")
            ones = persist.tile([128, 1], BF16, name="ones")
            nc.vector.memset(ones[:], 1.0)

            # ---------------- projection phase ----------------
            with tc.tile_pool(name="wts", bufs=1) as wts, \
                 tc.tile_pool(name="xblk", bufs=2) as xblk, \
                 tc.tile_pool(name="rt", bufs=2) as rt, \
                 tc.tile_pool(name="pp", bufs=2, space="PSUM") as pp:
                wqs = {k: wts.tile([128, DC, C], BF16, name=f"wq{k}")
                       for k in ("r", "a", "b")}
                wks = {k: wts.tile([128, DC, C], BF16, name=f"wk{k}")
                       for k in ("r", "a", "b")}
                wvs = {k: wts.tile([128, DC, C], BF16, name=f"wv{k}")
                       for k in ("r", "a", "b")}
                cos = wts.tile([128, T], BF16, name="cos")
                sin = wts.tile([128, T], BF16, name="sin")
                # weights + tables on the ACT ring, ordered by first use
                for k in ("a", "b", "r"):
                    nc.scalar.dma_start(wqs[k][:], wq[k][:])
                nc.scalar.dma_start(cos[:], cos_d[:])
                nc.scalar.dma_start(sin[:], sin_d[:])
                for k in ("a", "b", "r"):
                    nc.scalar.dma_start(wks[k][:], wk[k][:])
                for k in ("a", "b", "r"):
                    nc.scalar.dma_start(wvs[k][:], wv[k][:])

                for t5 in range(T5):
                    xi_t = xblk.tile([128, DC, 512], BF16, name="xi")
                    xr_t = xblk.tile([128, DC, 512], BF16, name="xr")
                    sx_t = xblk.tile([128, DC, 512], BF16, name="sx")
                    nc.sync.dma_start(xi_t[:], xi_d[:, t5])
                    nc.sync.dma_start(xr_t[:], xr_d[:, t5])
                    nc.vector.tensor_add(sx_t[:], xr_t[:], xi_t[:])
                    tsl = ts(t5, 512)

                    # q/k transposed [c, t] + RoPE into qkcat
                    for wsrc, hbase in ((wqs, 0), (wks, HC)):
                        for cc in range(2):
                            h0, h1 = hbase + 2 * cc, hbase + 2 * cc + 1
                            csl = ts(cc, 128)
                            trio = pp.tile([128, 1536], F32, name="trio")
                            m2, m3, m1 = (trio[:, 0:512], trio[:, 512:1024],
                                          trio[:, 1024:1536])
                            for dc in range(DC):
                                nc.tensor.matmul(
                                    m2, lhsT=wsrc["a"][:, dc, csl],
                                    rhs=xi_t[:, dc, :],
                                    start=(dc == 0), stop=(dc == DC - 1))
                            for dc in range(DC):
                                nc.tensor.matmul(
                                    m3, lhsT=wsrc["b"][:, dc, csl],
                                    rhs=xr_t[:, dc, :],
                                    start=(dc == 0), stop=(dc == DC - 1))
                            for dc in range(DC):
                                nc.tensor.matmul(
                                    m1, lhsT=wsrc["r"][:, dc, csl],
                                    rhs=sx_t[:, dc, :],
                                    start=(dc == 0), stop=(dc == DC - 1))
                            m1s = rt.tile([128, 512], BF16, name="m1s")
                            nc.scalar.copy(m1s[:], m1)
                            tr = rt.tile([128, 512], BF16, name="tr")
                            ti = rt.tile([128, 512], BF16, name="ti")
                            nc.vector.tensor_sub(tr[:], m1s[:], m2)
                            nc.vector.tensor_add(ti[:], m1s[:], m3)
                            t1 = rt.tile([128, 512], BF16, name="t1")
                            t2 = rt.tile([128, 512], BF16, name="t2")
                            t3 = rt.tile([128, 512], BF16, name="t3")
                            t4 = rt.tile([128, 512], BF16, name="t4")
                            nc.vector.tensor_mul(t1[:], tr[:], cos[:, tsl])
                            nc.vector.tensor_mul(t2[:], ti[:], sin[:, tsl])
                            nc.vector.tensor_mul(t3[:], tr[:], sin[:, tsl])
                            nc.vector.tensor_mul(t4[:], ti[:], cos[:, tsl])
                            nc.vector.tensor_sub(qkcat[0:64, h0, tsl],
                                                 t1[0:64, :], t2[0:64, :])
                            nc.vector.tensor_sub(qkcat[0:64, h1, tsl],
                                                 t1[64:128, :], t2[64:128, :])
                            nc.vector.tensor_add(qkcat[64:128, h0, tsl],
                                                 t3[0:64, :], t4[0:64, :])
                            nc.vector.tensor_add(qkcat[64:128, h1, tsl],
                                                 t3[64:128, :], t4[64:128, :])

                    # v natural [t, c] for the 4 tq chunks of this t5
                    for tql in range(4):
                        tq = 4 * t5 + tql
                        qsl = ts(tql, 128)
                        vt = pp.tile([128, 1536], F32, name="trio")
                        v2, v3, v1 = (vt[:, 0:256], vt[:, 512:768],
                                      vt[:, 1024:1280])
                        for dc in range(DC):
                            nc.tensor.matmul(v2, lhsT=xi_t[:, dc, qsl],
                                             rhs=wvs["a"][:, dc, :],
                                             start=(dc == 0),
                                             stop=(dc == DC - 1))
                        for dc in range(DC):
                            nc.tensor.matmul(v3, lhsT=xr_t[:, dc, qsl],
                                             rhs=wvs["b"][:, dc, :],
                                             start=(dc == 0),
                                             stop=(dc == DC - 1))
                        for dc in range(DC):
                            nc.tensor.matmul(v1, lhsT=sx_t[:, dc, qsl],
                                             rhs=wvs["r"][:, dc, :],
                                             start=(dc == 0),
                                             stop=(dc == DC - 1))
                        v1s = rt.tile([128, 256], BF16, name="v1s")
                        nc.scalar.copy(v1s[:], v1)
                        vre = v1s[:].rearrange("p (h d) -> p h d", h=HC)
                        v2e = v2.rearrange("p (h d) -> p h d", h=HC)
                        v3e = v3.rearrange("p (h d) -> p h d", h=HC)
                        nc.vector.tensor_sub(vcat[:, tq, :, 0:64], vre, v2e)
                        nc.vector.tensor_add(vcat[:, tq, :, 64:128], vre, v3e)

            # ---------------- attention phase ----------------
            with tc.tile_pool(name="ox", bufs=1) as ox:
                # prefetch o-projection weight combos on the idle sync ring
                ows = {k: ox.tile([128, 2, D], BF16, name=f"ow{k}")
                       for k in ("r", "i", "n")}
                for k in ("r", "i", "n"):
                    nc.sync.dma_start(ows[k][:], ow[k][:])

                with tc.tile_pool(name="att", bufs=12) as att, \
                     tc.tile_pool(name="sm", bufs=2) as sm, \
                     tc.tile_pool(name="attsm", bufs=2) as attsm, \
                     tc.tile_pool(name="sp", bufs=2, space="PSUM") as sp, \
                     tc.tile_pool(name="avp", bufs=2, space="PSUM") as avp:
                    from concourse import bass_isa
                    slots = [(iw, h) for iw in range(TW) for h in range(HC)]
                    es_tiles = {}
                    av_tiles = {}
                    sA_tiles = {}

                    def emit_s(si, jc):
                        iw, h = slots[si]
                        s = sp.tile([128, 1024], F32, name="s")
                        for half in range(2):
                            psl = ts(half, 512)
                            nc.tensor.matmul(
                                s[:, psl],
                                lhsT=qkcat[:, HC + h, ts(jc, 128)],
                                rhs=qkcat[:, h, ts(2 * iw + half, 512)],
                                start=True, stop=True)
                        es = att.tile([128, 1024], BF16, name="es")
                        nc.scalar.activation(es[:], s[:], EXP, scale=0.125)
                        es_tiles[(si, jc)] = es

                    pending = {}

                    def emit_av(si, jc):
                        iw, h = slots[si]
                        es = es_tiles.pop((si, jc))
                        if jc == 0:
                            av_tiles[si] = avp.tile([128, 1024], F32,
                                                    name="av")
                        av = av_tiles[si]
                        for half in range(2):
                            psl = ts(half, 512)
                            nc.tensor.matmul(av[:, psl],
                                             lhsT=vcat[:, jc, h, :],
                                             rhs=es[:, psl],
                                             start=(jc == 0),
                                             stop=(jc == TQ - 1))
                        if jc == 0:
                            sA_tiles[si] = sm.tile([128, 1024], BF16,
                                                   name="sA")
                            nc.vector.tensor_copy(sA_tiles[si][:], es[:])
                        else:
                            sA = sA_tiles[si]
                            nc.vector.tensor_add(sA[:], sA[:], es[:])
                        if jc == 8 and (si - 1) in pending:
                            finish_slot(si - 1)
                        if jc == TQ - 1:
                            # launch the (slow) partition reduction now and
                            # evacuate av to SBUF so its PSUM bank frees
                            # immediately; reciprocal + normalize run
                            # mid-next-head once the gpsimd latency is over
                            sA = sA_tiles.pop(si)
                            dnb = attsm.tile([128, 1024], F32, name="dnb")
                            nc.gpsimd.partition_all_reduce(
                                dnb[:], sA[:], channels=128,
                                reduce_op=bass_isa.ReduceOp.add)
                            avs = sm.tile([128, 1024], F32, name="avs")
                            nc.vector.tensor_copy(avs[:], av_tiles.pop(si)[:])
                            pending[si] = (avs, dnb)

                    def finish_slot(si):
                        iw, h = slots[si]
                        isl = ts(iw, 1024)
                        ucc, up0 = h // 2, (h % 2) * 64
                        avs, dnb = pending.pop(si)
                        rec = attsm.tile([128, 1024], F32, name="rec")
                        nc.vector.reciprocal_approx_fast(rec[:], dnb[:])
                        nc.vector.tensor_mul(urt[up0:up0 + 64, ucc, isl],
                                             avs[0:64, :], rec[0:64, :])
                        nc.vector.tensor_mul(uit[up0:up0 + 64, ucc, isl],
                                             avs[64:128, :], rec[64:128, :])

                    LAG = 3
                    stream = [(si, jc) for si in range(len(slots))
                              for jc in range(TQ)]
                    for pos, (si, jc) in enumerate(stream):
                        emit_s(si, jc)
                        if pos >= LAG:
                            emit_av(*stream[pos - LAG])
                    for pos in range(len(stream) - LAG, len(stream)):
                        emit_av(*stream[pos])
                    finish_slot(len(slots) - 1)

                # ---------------- output projection ----------------
                with tc.tile_pool(name="ost", bufs=3) as ost, \
                     tc.tile_pool(name="op", bufs=2, space="PSUM") as op:
                    for tq in range(TQ):
                        tslq = ts(tq, 128)
                        for oc in range(2):
                            osl = ts(oc, 512)
                            pr = op.tile([128, 512], F32, name="pr")
                            pi = op.tile([128, 512], F32, name="pi")
                            nc.tensor.matmul(pr[:], lhsT=urt[:, 0, tslq],
                                             rhs=ows["r"][:, 0, osl],
                                             start=True, stop=False)
                            nc.tensor.matmul(pr[:], lhsT=urt[:, 1, tslq],
                                             rhs=ows["r"][:, 1, osl],
                                             start=False, stop=False)
                            nc.tensor.matmul(pr[:], lhsT=uit[:, 0, tslq],
                                             rhs=ows["n"][:, 0, osl],
                                             start=False, stop=False)
                            nc.tensor.matmul(pr[:], lhsT=uit[:, 1, tslq],
                                             rhs=ows["n"][:, 1, osl],
                                             start=False, stop=True)
                            nc.tensor.matmul(pi[:], lhsT=urt[:, 0, tslq],
                                             rhs=ows["i"][:, 0, osl],
                                             start=True, stop=False)
                            nc.tensor.matmul(pi[:], lhsT=urt[:, 1, tslq],
                                             rhs=ows["i"][:, 1, osl],
                                             start=False, stop=False)
                            nc.tensor.matmul(pi[:], lhsT=uit[:, 0, tslq],
                                             rhs=ows["r"][:, 0, osl],
                                             start=False, stop=False)
                            nc.tensor.matmul(pi[:], lhsT=uit[:, 1, tslq],
                                             rhs=ows["r"][:, 1, osl],
                                             start=False, stop=True)
                            str_ = ost.tile([128, 512], F32, name="str")
                            sti = ost.tile([128, 512], F32, name="sti")
                            nc.scalar.copy(str_[:], pr[:])
                            nc.vector.tensor_copy(sti[:], pi[:])
                            nc.sync.dma_start(outr_d[tslq, osl], str_[:])
                            nc.scalar.dma_start(outi_d[tslq, osl], sti[:])

    nc.compile()
    return nc


def _to_bf16_kxm(arr, parts=128):
    """[K, M] fp32 -> [128, K//128, M] bf16 with K split as (chunk, part)."""
    k, m = arr.shape
    out = arr.reshape(k // parts, parts, m).transpose(1, 0, 2)
    return np.ascontiguousarray(out.astype(ml_dtypes.bfloat16))


def _x_blocks(arr):
    """[T, D] fp32 -> transposed, t5-blocked [128, T5, DC, 512] bf16."""
    xt = arr.T.reshape(DC, 128, T5, 512).transpose(1, 2, 0, 3)
    return np.ascontiguousarray(xt.astype(ml_dtypes.bfloat16))


def _rope_tables():
    inv_freq = 1.0 / (10000.0 ** (np.arange(0, HD, 2, dtype=np.float64) / HD))
    invf64 = np.concatenate([inv_freq, inv_freq])          # [64]
    ang = invf64[:, None] * np.arange(T, dtype=np.float64)[None, :]  # [64, T]
    cos2 = np.tile(np.cos(ang), (2, 1)).astype(ml_dtypes.bfloat16)
    sin2 = np.tile(np.sin(ang), (2, 1)).astype(ml_dtypes.bfloat16)
    return np.ascontiguousarray(cos2), np.ascontiguousarray(sin2)


def kernel(x_real, x_imag, q_wr, q_wi, k_wr, k_wi, v_wr, v_wi, o_wr, o_wi):
    global _COMPILED, LAST_RESULTS
    if _COMPILED is None:
        _COMPILED = _build()
    nc = _COMPILED

    cos2, sin2 = _rope_tables()
    xt = {}
    for b in range(B):
        xt[("r", b)] = _x_blocks(np.asarray(x_real[b], np.float32))
        xt[("i", b)] = _x_blocks(np.asarray(x_imag[b], np.float32))

    in_maps = []
    for core in range(NCORE):
        b, g = core // TP, core % TP
        cols = slice(g * C, (g + 1) * C)
        m = {"xrT": xt[("r", b)], "xiT": xt[("i", b)],
             "cos2": cos2, "sin2": sin2}
        for nm, wr_, wi_ in (("wq", q_wr, q_wi), ("wk", k_wr, k_wi),
                             ("wv", v_wr, v_wi)):
            wr_c = np.asarray(wr_[:, cols], np.float32)
            wi_c = np.asarray(wi_[:, cols], np.float32)
            m[f"{nm}_r"] = _to_bf16_kxm(wr_c)
            m[f"{nm}_a"] = _to_bf16_kxm(wr_c + wi_c)
            m[f"{nm}_b"] = _to_bf16_kxm(wi_c - wr_c)
        owr_c = np.asarray(o_wr[cols, :], np.float32)
        owi_c = np.asarray(o_wi[cols, :], np.float32)
        m["ow_r"] = _to_bf16_kxm(owr_c)
        m["ow_i"] = _to_bf16_kxm(owi_c)
        m["ow_n"] = _to_bf16_kxm(-owi_c)
        in_maps.append(m)

    res = run_bass_kernel_spmd(nc, in_maps, core_ids=list(range(NCORE)))
    LAST_RESULTS = res

    final_r = np.zeros((B, T, D), np.float32)
    final_i = np.zeros((B, T, D), np.float32)
    for core in range(NCORE):
        b = core // TP
        final_r[b] += res.results[core]["out_r"]
        final_i[b] += res.results[core]["out_i"]
    return final_r, final_i
